# revision 53
# baseline (speedup 1.0000x reference)
"""Trainium2 Bass kernel: JointSpatioTemporalAttention, bf16 matmuls, 8-core SPMD.

Sharding: core c = (b, g) with b = c // 4 (batch), g = c % 4 (KV group).
Each core computes q-heads [4g, 4g+4) and kv-head g for batch b end-to-end
(QKV proj -> RMSNorm -> 3D RoPE -> attention -> partial out-proj), returning
the partial y^T = Wo[256g:256g+256, :].T @ O_norm^T in bf16.  Host sums the
4 group partials per batch (fp32) and adds bo.  No on-device collectives.

v4 design: the ~142us of FD=1024 EXPs on the ACT engine is the hard floor;
everything else is arranged to hide under it.
  - Phase A is split: a KV-only pass (K/V projection + K norm/rope +
    transposes) runs first so attention can start early; the Q pass runs in
    groups of 4 token tiles, group 0 before attention and groups 1-3
    STRIPED between attention quarters (PE slack absorbs them).
  - RMSNorm rinv = exp(-0.5*ln(ms)) via Ln+Exp so the ONLY ACT table set
    used anywhere is natural_log_exp_and_others (Copy/Ln/Exp) -> one
    ACT_TABLE_LOAD, no mid-stream set thrash.
  - Scores are ROW-TILED via tile_position: head pair stacked on partitions
    0:64/64:128 (QT2), K^T duplicated (kT2); the two K=64 matmuls run
    concurrently on different PE row-groups.
  - Work unit = (hh, pr, qq, mc): S^T psum [128,1024] = pair x 512 queries,
    one FD=1024 exp per unit, double-buffered -> ACT back-to-back.
  - PV: V||ones stationary (shared by the pair), O' [65,2,512] accumulated
    over 16 key chunks; denominator in row 64.  Epilogue split: O'->SBUF
    copy inline; transpose/normalize (finisher) deferred to the next
    half's stripe windows / tail.
  - B2 (O_nat->oT) + C (out-proj) for hh=0 striped into hh=1's attention;
    hh=1's run in a tail with fat PSUM pools; input DMA split across the
    sync+scalar HWDGE queues; outputs bf16.
"""

import os
import sys
from contextlib import ExitStack

import numpy as np

for _p in ("/opt/trn_rl_repo",):
    if _p not in sys.path:
        sys.path.append(_p)

import concourse.bass as bass  # noqa: E402
import concourse.mybir as mybir  # noqa: E402
import concourse.tile as tile  # noqa: E402
from concourse import bacc  # noqa: E402
from concourse.bass_utils import run_bass_kernel_spmd  # noqa: E402

import ml_dtypes  # noqa: E402

F32 = mybir.dt.float32
BF16 = mybir.dt.bfloat16
AF = mybir.ActivationFunctionType
ALU = mybir.AluOpType
AX = mybir.AxisListType

P = 128
B = 2
N = 2048          # tokens = 8*16*16
D = 1024
HD = 64           # head dim
NHEADS = 16
NKV = 4
CQ = 256          # q cols per core (4 heads)
CK = 64           # k/v cols per core (1 kv head)
CQK = CQ + CK     # 320
CQKV = CQK + CK   # 384
NT = N // P       # 16 token tiles
KD = D // P       # 8 contraction chunks
T_, H_, W_ = 8, 16, 16
THETA = 10000.0
EPS = 1e-6
NCORES = 8
NH = N // 2       # 1024: query half

_PROGRAM = None
LAST_RESULTS = None


def _emit(ctx: ExitStack, tc: "tile.TileContext"):
    nc = tc.nc
    f = F32
    bf = BF16

    xT = nc.dram_tensor("xT", [D, N], bf, kind="ExternalInput").ap()
    wqkv = nc.dram_tensor("wqkv", [P, KD, CQKV], bf, kind="ExternalInput").ap()
    wo = nc.dram_tensor("wo", [CQ, D], bf, kind="ExternalInput").ap()
    csd = nc.dram_tensor("cs5", [P, NT, 2, CQK], bf, kind="ExternalInput").ap()
    identd = nc.dram_tensor("ident", [P, P], bf, kind="ExternalInput").ap()
    yT = nc.dram_tensor("yT", [D, N], bf, kind="ExternalOutput").ap()

    # ---------------- persistent pools ----------------
    constp = ctx.enter_context(tc.tile_pool(name="const", bufs=1))
    epsb = constp.tile([P, 1], f, tag="epsb")
    nc.vector.memset(epsb[:], EPS)

    persist = ctx.enter_context(tc.tile_pool(name="persist", bufs=1))
    QT2 = persist.tile([P, 2, N], bf, tag="QT2")
    kT2 = persist.tile([P, N], bf, tag="kT2")
    # duplicate of K^T on partitions 64:128, ONE TILE PER KEY CHUNK:
    # DMA-write -> read dependencies are tile-granular, so a shared tile
    # would make every score matmul wait for the LAST dup DMA
    kTd = persist.tile([P, N], bf, tag="kTd")
    v_nat = persist.tile([P, NT, HD + 1], bf, tag="v_nat")
    nc.vector.memset(v_nat[:, :, HD:HD + 1], 1.0)
    O_nat = persist.tile([P, NT, CQ], bf, tag="O_nat")
    oT = persist.tile([P, 2, N], bf, tag="oT")
    cs_all = persist.tile([P, NT, 2, CQK], bf, tag="cs_all")
    ssk_all = persist.tile([P, NT], f, tag="ssk_all")
    rik_all = persist.tile([P, NT], f, tag="rik_all")
    ssq_all = persist.tile([P, NT, 4], f, tag="ssq_all")
    riq_all = persist.tile([P, NT, 4], f, tag="riq_all")

    # ---- input DMA: sync queue = weights + xT; scalar queue = cos/sin ----
    xwp = ctx.enter_context(tc.tile_pool(name="xw", bufs=1))
    w_sb = xwp.tile([P, KD, CQKV], bf, tag="wqkv")
    nc.sync.dma_start(out=w_sb[:], in_=wqkv)
    # cos/sin host-packed per-partition-contiguous (20KB rows): one DMA
    nc.gpsimd.dma_start(out=cs_all[:], in_=csd)
    # xT: even k-chunks on sync, odd on scalar, quarter-major.  SEPARATE
    # tiles per queue: cross-queue writes into one tile serialize through
    # Tile's (tile-granular) dependency tracking, adding ~1.5-3us per DMA.
    xT_e = xwp.tile([P, KD // 2, N], bf, tag="xT_e")
    xT_o = xwp.tile([P, KD // 2, N], bf, tag="xT_o")
    xT_t = xT.rearrange("(k p) n -> k p n", p=P)

    def _xt(k):
        return (xT_e if k % 2 == 0 else xT_o)[:, k // 2, :]

    for k in range(KD):
        eng = nc.sync if k % 2 == 0 else nc.scalar
        eng.dma_start(out=_xt(k), in_=xT_t[k])
    ident = constp.tile([P, P], bf, tag="ident")
    nc.sync.dma_start(out=ident[:], in_=identd)
    wo_sb = constp.tile([P, 2, D], bf, tag="wo")
    wo_t = wo.rearrange("(k p) d -> k p d", p=P)
    for k2 in range(2):
        nc.scalar.dma_start(out=wo_sb[:, k2, :], in_=wo_t[k2])

    # B-phase pools enter FIRST (tile pools are a strict LIFO stack; these
    # outlive the KV/Q pools).  PSUM budget: psS 4 + psO 2 = 6 banks held
    # throughout; KV/Q phases use the remaining 2.
    scale = float(HD) ** -0.5
    yT_t = yT.rearrange("(t p) n -> t p n", p=P)
    otp = ctx.enter_context(tc.tile_pool(name="ot", bufs=9))
    rrp = ctx.enter_context(tc.tile_pool(name="rr", bufs=2))
    ysbp = ctx.enter_context(tc.tile_pool(name="ysb", bufs=9))
    es_b = ExitStack()
    psS = es_b.enter_context(tc.tile_pool(name="psS", bufs=2, space="PSUM"))
    psO = es_b.enter_context(tc.tile_pool(name="psO", bufs=1, space="PSUM"))
    pTp = es_b.enter_context(tc.tile_pool(name="pT", bufs=4))
    es_qsb = ExitStack()
    qnp = es_qsb.enter_context(tc.tile_pool(name="qnp", bufs=6))
    smq = es_qsb.enter_context(tc.tile_pool(name="smq", bufs=4))

    kn_t = {}
    qn_t = {}
    pq_t = {}
    otm = {}
    ysb_tiles = {}

    def q_lnexp_job(g):
        gsl = slice(g * 4, g * 4 + 4)
        lnq = smq.tile([P, 4, 4], f, tag="lnq", name="lnq")
        nc.scalar.activation(lnq[:], ssq_all[:, gsl, :], AF.Ln,
                             bias=epsb[:], scale=1.0 / HD)
        nc.scalar.activation(riq_all[:, gsl, :], lnq[:], AF.Exp, scale=-0.5)

    def q_rope_job(i, pool, tag):
        nsl = slice(i * P, (i + 1) * P)
        qn = qn_t.pop(i)
        qc = smq.tile([P, CQ], bf, tag="qc", name="qc")
        nc.gpsimd.tensor_tensor(qc[:], qn[:], cs_all[:, i, 0, 0:CQ], op=ALU.mult)
        rsq = smq.tile([P, CQ], bf, tag="rsq", name="rsq")
        qn_sw = qn.rearrange("p (j s) -> p j s", s=2)[:, :, ::-1]
        nc.gpsimd.tensor_tensor(rsq[:], qn_sw, cs_all[:, i, 1, 0:CQ], op=ALU.mult)
        qf = smq.tile([P, CQ], bf, tag="qf", name="qf")
        nc.vector.tensor_tensor(qf[:], qc[:], rsq[:], op=ALU.add)
        qf4 = qf.rearrange("p (h d) -> p h d", d=HD)
        nc.vector.tensor_mul(qf4, qf4, riq_all[:, i, :].to_broadcast((P, 4, HD)))
        ptq = pool.tile([P, CQ], bf, tag=tag, name="ptq")
        nc.tensor.transpose(ptq[:, 0:P], qf[:, 0:P], ident[:])
        nc.tensor.transpose(ptq[:, P:CQ], qf[:, P:CQ], ident[:])
        nc.vector.tensor_copy(
            QT2[:, :, nsl], ptq.rearrange("p (h n) -> p h n", h=2))

    def emit_unit(hh, pr, qq, mc, ops):
        qcol = hh * NH + qq * 512
        msl = slice(mc * P, (mc + 1) * P)
        sps = psS.tile([P, 1024], f, tag="psS", name="sps")
        nc.tensor.matmul(
            sps[:, 0:512],
            lhsT=kT2[0:CK, msl],
            rhs=QT2[0:CK, pr, qcol:qcol + 512],
            start=True, stop=True, tile_position=(0, 0),
        )
        nc.tensor.matmul(
            sps[:, 512:1024],
            lhsT=kTd[CK:2 * CK, msl],
            rhs=QT2[CK:2 * CK, pr, qcol:qcol + 512],
            start=True, stop=True, tile_position=(CK, 0),
        )
        pTt = pTp.tile([P, 1024], bf, tag="pT", name="pTt")
        nc.scalar.activation(pTt[:], sps[:], AF.Exp, scale=scale)
        for s in range(2):
            nc.tensor.matmul(
                ops[:, s, :],
                lhsT=v_nat[:, mc, :],
                rhs=pTt[:, s * 512:(s + 1) * 512],
                start=(mc == 0),
                stop=(mc == NT - 1),
            )

    # ================ phase KV: K/V proj + K norm/rope + K^T ================
    # Quarter (0,0,0)'s attention units are WOVEN into KV groups 1-3: unit
    # mc is emitted right after KV tile mc+4 finishes its K^T, so the exp
    # stream starts while the (DMA-gated) KV pass is still running.
    with tc.tile_pool(name="psKV", bufs=1, space="PSUM") as psKV, \
         tc.tile_pool(name="psTK", bufs=1, space="PSUM") as psTK, \
         tc.tile_pool(name="knp", bufs=6) as knp, \
         tc.tile_pool(name="smk", bufs=4) as smk:
        for g in range(4):
            tiles = range(g * 4, g * 4 + 4)
            for i in tiles:
                nsl = slice(i * P, (i + 1) * P)
                # tiles 0-3: fused full-width QKV projection (Q lands here
                # too, so qgroup(0) needs no projection of its own)
                fused = i < 4
                wlo = 0 if fused else CQ
                pkv = psKV.tile([P, CQKV], f, tag="pkv", name="pkv")
                for k in range(KD):
                    nc.tensor.matmul(
                        pkv[:, 0:CQKV - wlo],
                        lhsT=_xt(k)[:, nsl], rhs=w_sb[:, k, wlo:CQKV],
                        start=(k == 0), stop=(k == KD - 1),
                    )
                kn = knp.tile([P, CK], bf, tag="kn", name="kn")
                kn_t[i] = kn
                nc.vector.tensor_copy(kn[:], pkv[:, CQ - wlo:CQ - wlo + CK])
                nc.vector.tensor_copy(v_nat[:, i, 0:HD],
                                      pkv[:, CQ - wlo + CK:CQKV - wlo])
                if fused:
                    qn = qnp.tile([P, CQ], bf, tag="qn", name="qn")
                    qn_t[i] = qn
                    nc.vector.tensor_copy(qn[:], pkv[:, 0:CQ])
                    sq = smk.tile([P, CQ], bf, tag="sq", name="sq")
                    nc.gpsimd.tensor_tensor(sq[:], qn[:], qn[:], op=ALU.mult)
                    nc.vector.tensor_reduce(
                        ssq_all[:, i, :],
                        sq.rearrange("p (h d) -> p h d", d=HD),
                        axis=AX.X, op=ALU.add,
                    )
                sqk = smk.tile([P, CK], bf, tag="sqk")
                nc.vector.tensor_tensor(sqk[:], kn[:], kn[:], op=ALU.mult)
                nc.vector.tensor_reduce(
                    ssk_all[:, i:i + 1],
                    sqk.rearrange("p (h d) -> p h d", d=CK), axis=AX.X, op=ALU.add,
                )
            gsl = slice(g * 4, g * 4 + 4)
            lnk = smk.tile([P, 4], f, tag="lnk")
            nc.scalar.activation(lnk[:], ssk_all[:, gsl], AF.Ln,
                                 bias=epsb[:], scale=1.0 / HD)
            nc.scalar.activation(rik_all[:, gsl], lnk[:], AF.Exp, scale=-0.5)
            for i in tiles:
                nsl = slice(i * P, (i + 1) * P)
                kn = kn_t.pop(i)
                kc = smk.tile([P, CK], bf, tag="kc")
                nc.gpsimd.tensor_tensor(kc[:], kn[:], cs_all[:, i, 0, 2 * P:CQK],
                                        op=ALU.mult)
                rsk = smk.tile([P, CK], bf, tag="rsk")
                kn_sw = kn.rearrange("p (j s) -> p j s", s=2)[:, :, ::-1]
                nc.gpsimd.tensor_tensor(rsk[:], kn_sw, cs_all[:, i, 1, 2 * P:CQK],
                                        op=ALU.mult)
                kf = smk.tile([P, CK], bf, tag="kf")
                nc.vector.tensor_tensor(kf[:], kc[:], rsk[:], op=ALU.add)
                kf1 = kf.rearrange("p (h d) -> p h d", d=CK)
                nc.vector.tensor_mul(
                    kf1, kf1, rik_all[:, i:i + 1].to_broadcast((P, 1, CK)))
                ptk = psTK.tile([P, CQ], bf, tag="ptk", name="ptk")
                nc.tensor.transpose(ptk[0:CK, 0:P], kf[:], ident[:])
                nc.vector.tensor_copy(kT2[0:CK, nsl], ptk[0:CK, 0:P])
                # duplicate this K^T tile onto partitions 64:128 (gpsimd q)
                nc.scalar.dma_start(out=kTd[CK:2 * CK, nsl],
                                    in_=kT2[0:CK, nsl])
                if g >= 1:
                    emit_unit(0, 0, 0, i - 4, opsA)
            if g == 0:
                # group-0 Q rope (projection was fused); enables quarter A
                q_lnexp_job(0)
                for i in tiles:
                    q_rope_job(i, psTK, "ptk")
                opsA = psO.tile([HD + 1, 2, 512], f, tag="psO", name="opsA")

    # quarter A's last 4 units + epilogue copy
    for mc in range(12, NT):
        emit_unit(0, 0, 0, mc, opsA)
    o_tmpA = otp.tile([HD + 1, 2, 512], bf, tag="o_tmp", name="o_tmpA")
    nc.vector.tensor_copy(o_tmpA[:], opsA[:])
    otm[(0, 0, 0)] = o_tmpA

    # ---------------- Q pass (grouped; groups 1-3 striped into phase B) ----
    es_q = ExitStack()
    psQ = es_q.enter_context(tc.tile_pool(name="psQ", bufs=1, space="PSUM"))
    psTQ = es_q.enter_context(tc.tile_pool(name="psTQ", bufs=1, space="PSUM"))

    def q_tile_job_a(i):
        nsl = slice(i * P, (i + 1) * P)
        pq = psQ.tile([P, CQ], f, tag="pq", name="pq")
        pq_t[i] = pq
        for k in range(4):
            nc.tensor.matmul(
                pq[:], lhsT=_xt(k)[:, nsl], rhs=w_sb[:, k, 0:CQ],
                start=(k == 0), stop=False,
            )

    def q_tile_job_b(i):
        nsl = slice(i * P, (i + 1) * P)
        pq = pq_t.pop(i)
        for k in range(4, KD):
            nc.tensor.matmul(
                pq[:], lhsT=_xt(k)[:, nsl], rhs=w_sb[:, k, 0:CQ],
                start=False, stop=(k == KD - 1),
            )
        qn = qnp.tile([P, CQ], bf, tag="qn", name="qn")
        qn_t[i] = qn
        nc.vector.tensor_copy(qn[:], pq[:])
        sq = smq.tile([P, CQ], bf, tag="sq", name="sq")
        nc.vector.tensor_tensor(sq[:], qn[:], qn[:], op=ALU.mult)
        nc.vector.tensor_reduce(
            ssq_all[:, i, :],
            sq.rearrange("p (h d) -> p h d", d=HD), axis=AX.X, op=ALU.add,
        )

    def qgroup_jobs(g):
        tiles = list(range(g * 4, g * 4 + 4))
        jobs = []
        for i in tiles:
            jobs.append(lambda i=i: q_tile_job_a(i))
            jobs.append(lambda i=i: q_tile_job_b(i))
        jobs.append(lambda: q_lnexp_job(g))
        jobs += [(lambda i=i: q_rope_job(i, psTQ, "ptq")) for i in tiles]
        return jobs

    # ================ phase B ================

    def attn_quarter(hh, pr, qq, side=()):
        # side: list of closures, one emitted after each unit's instructions
        # so striped work interleaves finely with the ACT-bound exp stream.
        side = list(side)
        ops = psO.tile([HD + 1, 2, 512], f, tag="psO", name="ops")
        for mc in range(NT):
            emit_unit(hh, pr, qq, mc, ops)
            if mc < len(side):
                side[mc]()
        for jb in side[NT:]:
            jb()
        o_tmp = otp.tile([HD + 1, 2, 512], bf, tag="o_tmp", name="o_tmp")
        nc.vector.tensor_copy(o_tmp[:], ops[:])
        otm[(hh, pr, qq)] = o_tmp

    def finisher(hh, pr, qq, scrp):
        # O'^T (SBUF) -> natural via PE transpose, normalize by 1/denom
        o_tmp = otm.pop((hh, pr, qq))
        tbase = hh * 8 + qq * 4
        for s in range(2):
            scr = scrp.tile([P, 4, 68], bf, tag="scr", name="scr")
            for qb in range(4):
                nc.tensor.transpose(
                    scr[:, qb, 0:HD + 1],
                    o_tmp[:, s, qb * P:(qb + 1) * P],
                    ident[0:HD + 1, 0:HD + 1],
                )
            rin = rrp.tile([P, 4], f, tag="rin", name="rin")
            nc.vector.reciprocal(rin[:], scr[:, :, HD])
            h = 2 * pr + s
            nc.vector.tensor_mul(
                O_nat[:, tbase:tbase + 4, h * HD:(h + 1) * HD],
                scr[:, :, 0:HD],
                rin.to_broadcast((P, 4, HD)),
            )

    def b2_job(j, k2, scrp):
        scr = scrp.tile([P, 4, 68], bf, tag="scr", name="scr")
        ptu = scr.rearrange("p a b -> p (a b)")[:, 0:P]
        nc.tensor.transpose(ptu, O_nat[:, j, k2 * P:(k2 + 1) * P], ident[:])
        nc.vector.tensor_copy(oT[:, k2, j * P:(j + 1) * P], ptu)

    def c_job(hh, mt, sg, yp, on_scalar, dma_eng):
        yps = yp.tile([P, 512], f, tag="yps", name="yps")
        qsl = slice(hh * NH + sg * 512, hh * NH + (sg + 1) * 512)
        for k2 in range(2):
            nc.tensor.matmul(
                yps[:],
                lhsT=wo_sb[:, k2, mt * P:(mt + 1) * P],
                rhs=oT[:, k2, qsl],
                start=(k2 == 0),
                stop=(k2 == 1),
            )
        if sg == 0:
            ysb_tiles[(hh, mt)] = ysbp.tile([P, 2, 512], bf, tag="ysb",
                                            name="ysb")
        ysb = ysb_tiles[(hh, mt)]
        if on_scalar:
            nc.scalar.copy(ysb[:, sg, :], yps[:])
        else:
            nc.vector.tensor_copy(ysb[:, sg, :], yps[:])
        if sg == 1:
            dma_eng.dma_start(
                out=yT_t[mt][:, hh * NH:(hh + 1) * NH],
                in_=ysb.rearrange("p a b -> p (a b)"),
            )

    # ---- schedule: quarters qq-outer so each striped Q group has a full
    # quarter of slack before its output is consumed; hh0's epilogue work
    # striped into late-hh0/hh1 quarters; hh1's qq0 epilogue striped into
    # the qq1 quarters; only the qq1 epilogue remains as a true tail. ----
    # quarter (0,0,0) already ran, woven into the KV pass
    attn_quarter(0, 1, 0, side=qgroup_jobs(1))            # tiles 4-7 (for C)
    attn_quarter(0, 0, 1, side=qgroup_jobs(2) + qgroup_jobs(3))
    es_q.close()
    es_qsb.close()
    es_s = ExitStack()
    psScr = es_s.enter_context(tc.tile_pool(name="psScr", bufs=1, space="PSUM"))
    psY = es_s.enter_context(tc.tile_pool(name="psY", bufs=1, space="PSUM"))

    side_D = ([lambda: finisher(0, 0, 0, psScr), lambda: finisher(0, 1, 0, psScr)]
              + [(lambda j=j, k2=k2: b2_job(j, k2, psScr))
                 for j in range(0, 4) for k2 in range(2)])
    attn_quarter(0, 1, 1, side=side_D)

    side_E = ([lambda: finisher(0, 0, 1, psScr), lambda: finisher(0, 1, 1, psScr)]
              + [(lambda j=j, k2=k2: b2_job(j, k2, psScr))
                 for j in range(4, 8) for k2 in range(2)]
              + [(lambda sg=sg: c_job(0, 0, sg, psY, False, nc.sync))
                 for sg in range(2)])
    attn_quarter(1, 0, 0, side=side_E)

    side_F = [(lambda mt=mt, sg=sg: c_job(0, mt, sg, psY, False, nc.sync))
              for mt in range(1, 8) for sg in range(2)]
    attn_quarter(1, 1, 0, side=side_F)

    side_G = ([lambda: finisher(1, 0, 0, psScr), lambda: finisher(1, 1, 0, psScr)]
              + [(lambda j=j, k2=k2: b2_job(j, k2, psScr))
                 for j in range(8, 12) for k2 in range(2)])
    attn_quarter(1, 0, 1, side=side_G)

    side_H = ([(lambda mt=mt: c_job(1, mt, 0, psY, False, nc.sync))
               for mt in range(8)]
              + [lambda: finisher(1, 0, 1, psScr)]
              + [(lambda j=j: b2_job(j, 0, psScr)) for j in range(12, 16)])
    attn_quarter(1, 1, 1, side=side_H)

    # ---- tail: remaining hh1/qq1 epilogue with fat psum pools ----
    es_s.close()
    es_b.close()
    with tc.tile_pool(name="psScr2", bufs=3, space="PSUM") as psScr2, \
         tc.tile_pool(name="psY2", bufs=4, space="PSUM") as psY2:
        finisher(1, 1, 1, psScr2)
        for j in range(12, 16):
            b2_job(j, 1, psScr2)
        for mt in range(8):
            c_job(1, mt, 1, psY2, on_scalar=(mt % 2 == 0),
                  dma_eng=(nc.scalar if mt % 2 else nc.sync))


def _patch_act_tables(arch):
    """Reorder the (process-cached) activation-table dict so Exp, Ln and
    Copy all resolve to the single `natural_log_exp_and_others` set.  The
    default greedy per-function selection puts Exp in `exp_and_others` and
    Ln in `natural_log`, reloading ACT tables (~2.7us each) every time an
    RMSNorm Ln/Exp pair is interleaved with the softmax Exp stream."""
    from concourse.hw_specs import get_activation_tables
    t = get_activation_tables(arch)
    pref = "natural_log_exp_and_others"
    # Keep dict order/keys intact (set indices must match the real
    # act_info.json); just make `pref` the only set containing Exp/Ln.
    for name, fns in t.items():
        if name != pref:
            fns.discard(AF.Exp)
            fns.discard(AF.Ln)


def _build_program():
    global _PROGRAM
    if _PROGRAM is not None:
        return _PROGRAM
    nc = bacc.Bacc(
        "TRN2",
        target_bir_lowering=False,
        debug=False,
        enable_asserts=False,
        num_devices=NCORES,
    )
    _patch_act_tables(nc.m.arch)
    with tile.TileContext(nc) as tc:
        with ExitStack() as ctx:
            _emit(ctx, tc)
    nc.finalize()
    _PROGRAM = nc
    return nc


# ---------------- host-side RoPE/scale table construction ----------------

def _rope_cs(n, d):
    inv = 1.0 / (THETA ** (np.arange(0, d, 2, dtype=np.float32) / d))
    fr = np.arange(n, dtype=np.float32)[:, None] * inv[None, :]
    emb = np.concatenate([fr, fr], axis=-1)
    return np.cos(emb), np.sin(emb)


def _perm():
    dt = HD // 4
    dh = HD // 4
    dw = HD - dt - dh
    perm = np.empty(HD, np.int64)
    for off, sz in ((0, dt), (dt, dh), (dt + dh, dw)):
        m = sz // 2
        for j in range(m):
            perm[off + 2 * j] = off + j
            perm[off + 2 * j + 1] = off + m + j
    return perm


def _host_tables(qn_w, kn_w):
    dt = HD // 4
    dh = HD // 4
    dw = HD - dt - dh
    cos_t, sin_t = _rope_cs(T_, dt)
    cos_h, sin_h = _rope_cs(H_, dh)
    cos_w, sin_w = _rope_cs(W_, dw)
    tt = np.repeat(np.arange(T_), H_ * W_)
    hh = np.tile(np.repeat(np.arange(H_), W_), T_)
    ww = np.tile(np.arange(W_), T_ * H_)
    cos = np.empty((N, HD), np.float32)
    sin = np.empty((N, HD), np.float32)
    cos[:, 0:dt] = cos_t[tt]
    cos[:, dt:dt + dh] = cos_h[hh]
    cos[:, dt + dh:] = cos_w[ww]
    sin[:, 0:dt] = sin_t[tt]
    sin[:, dt:dt + dh] = sin_h[hh]
    sin[:, dt + dh:] = sin_w[ww]

    perm = _perm()
    cosP = cos[:, perm]
    sgn = np.empty(HD, np.float32)
    sgn[0::2] = -1.0
    sgn[1::2] = 1.0
    sgnsinP = sin[:, perm] * sgn[None, :]

    def fold(w):
        wp = np.asarray(w, np.float32)[perm]
        swp = wp.reshape(-1, 2)[:, ::-1].reshape(-1)
        return cosP * wp[None, :], sgnsinP * swp[None, :]

    cos_q, sin_q = fold(qn_w)
    cos_k, sin_k = fold(kn_w)
    cos5 = np.concatenate([np.tile(cos_q, (1, 4)), cos_k], axis=1)
    sin5 = np.concatenate([np.tile(sin_q, (1, 4)), sin_k], axis=1)
    return np.ascontiguousarray(cos5), np.ascontiguousarray(sin5)


def _bf16(a):
    return np.asarray(a, np.float32).astype(ml_dtypes.bfloat16)


def kernel(**inputs):
    global LAST_RESULTS
    x = np.asarray(inputs["x"], np.float32)
    Wq = np.asarray(inputs["Wq"], np.float32)
    Wk = np.asarray(inputs["Wk"], np.float32)
    Wv = np.asarray(inputs["Wv"], np.float32)
    Wo = np.asarray(inputs["Wo"], np.float32)
    bq = np.asarray(inputs["bq"], np.float32)
    bk = np.asarray(inputs["bk"], np.float32)
    bv = np.asarray(inputs["bv"], np.float32)
    bo = np.asarray(inputs["bo"], np.float32)
    qn_w = np.asarray(inputs["qn_w"], np.float32)
    kn_w = np.asarray(inputs["kn_w"], np.float32)

    assert x.shape == (B, N, D), x.shape
    # device program omits the qkv bias add (biases are zero in this model)
    assert not bq.any() and not bk.any() and not bv.any(), \
        "nonzero qkv bias unsupported by this kernel build"
    cos5, sin5 = _host_tables(qn_w, kn_w)
    perm = _perm()

    nc = _build_program()
    in_maps = []
    xT_b = [np.ascontiguousarray(_bf16(x[b]).T) for b in range(B)]
    identb = np.eye(P, dtype=ml_dtypes.bfloat16)
    csH = np.stack([cos5.reshape(NT, P, CQK), sin5.reshape(NT, P, CQK)],
                   axis=2).transpose(1, 0, 2, 3)   # [P, NT, 2, CQK]
    cs5b = np.ascontiguousarray(_bf16(csH))
    for c in range(NCORES):
        b, g = c // 4, c % 4
        wq_g = Wq[:, g * CQ:(g + 1) * CQ].reshape(D, 4, HD)[:, :, perm].reshape(D, CQ)
        wk_g = Wk[:, g * CK:(g + 1) * CK][:, perm]
        wv_g = Wv[:, g * CK:(g + 1) * CK]
        wqkv_ = np.concatenate([wq_g, wk_g, wv_g], axis=1)
        wH = _bf16(wqkv_).reshape(KD, P, CQKV).transpose(1, 0, 2)
        in_maps.append({
            "xT": xT_b[b],
            "wqkv": np.ascontiguousarray(wH),
            "wo": np.ascontiguousarray(_bf16(Wo[g * CQ:(g + 1) * CQ, :])),
            "cs5": cs5b,
            "ident": identb,
        })

    res = run_bass_kernel_spmd(nc, in_maps, list(range(NCORES)))
    LAST_RESULTS = res
    out = np.empty((B, N, D), np.float32)
    for b in range(B):
        acc = res.results[4 * b]["yT"].astype(np.float32)
        for g in range(1, 4):
            acc = acc + res.results[4 * b + g]["yT"].astype(np.float32)
        out[b] = acc.T + bo[None, :]
    return out


if __name__ == "__main__":
    nc = _build_program()
    print("built ok")


# revision 54
# speedup vs baseline: 1.0077x; 1.0077x over previous
"""Trainium2 Bass kernel: JointSpatioTemporalAttention, bf16 matmuls, 8-core SPMD.

Sharding: core c = (b, g) with b = c // 4 (batch), g = c % 4 (KV group).
Each core computes q-heads [4g, 4g+4) and kv-head g for batch b end-to-end
(QKV proj -> RMSNorm -> 3D RoPE -> attention -> partial out-proj), returning
the partial y^T = Wo[256g:256g+256, :].T @ O_norm^T in bf16.  Host sums the
4 group partials per batch (fp32) and adds bo.  No on-device collectives.

v4 design: the ~142us of FD=1024 EXPs on the ACT engine is the hard floor;
everything else is arranged to hide under it.
  - Phase A is split: a KV-only pass (K/V projection + K norm/rope +
    transposes) runs first so attention can start early; the Q pass runs in
    groups of 4 token tiles, group 0 before attention and groups 1-3
    STRIPED between attention quarters (PE slack absorbs them).
  - RMSNorm rinv = exp(-0.5*ln(ms)) via Ln+Exp so the ONLY ACT table set
    used anywhere is natural_log_exp_and_others (Copy/Ln/Exp) -> one
    ACT_TABLE_LOAD, no mid-stream set thrash.
  - Scores are ROW-TILED via tile_position: head pair stacked on partitions
    0:64/64:128 (QT2), K^T duplicated (kT2); the two K=64 matmuls run
    concurrently on different PE row-groups.
  - Work unit = (hh, pr, qq, mc): S^T psum [128,1024] = pair x 512 queries,
    one FD=1024 exp per unit, double-buffered -> ACT back-to-back.
  - PV: V||ones stationary (shared by the pair), O' [65,2,512] accumulated
    over 16 key chunks; denominator in row 64.  Epilogue split: O'->SBUF
    copy inline; transpose/normalize (finisher) deferred to the next
    half's stripe windows / tail.
  - B2 (O_nat->oT) + C (out-proj) for hh=0 striped into hh=1's attention;
    hh=1's run in a tail with fat PSUM pools; input DMA split across the
    sync+scalar HWDGE queues; outputs bf16.
"""

import os
import sys
from contextlib import ExitStack

import numpy as np

for _p in ("/opt/trn_rl_repo",):
    if _p not in sys.path:
        sys.path.append(_p)

import concourse.bass as bass  # noqa: E402
import concourse.mybir as mybir  # noqa: E402
import concourse.tile as tile  # noqa: E402
from concourse import bacc  # noqa: E402
from concourse.bass_utils import run_bass_kernel_spmd  # noqa: E402

import ml_dtypes  # noqa: E402

F32 = mybir.dt.float32
BF16 = mybir.dt.bfloat16
AF = mybir.ActivationFunctionType
ALU = mybir.AluOpType
AX = mybir.AxisListType

P = 128
B = 2
N = 2048          # tokens = 8*16*16
D = 1024
HD = 64           # head dim
NHEADS = 16
NKV = 4
CQ = 256          # q cols per core (4 heads)
CK = 64           # k/v cols per core (1 kv head)
CQK = CQ + CK     # 320
CQKV = CQK + CK   # 384
NT = N // P       # 16 token tiles
KD = D // P       # 8 contraction chunks
T_, H_, W_ = 8, 16, 16
THETA = 10000.0
EPS = 1e-6
NCORES = 8
NH = N // 2       # 1024: query half

_PROGRAM = None
LAST_RESULTS = None


def _emit(ctx: ExitStack, tc: "tile.TileContext"):
    nc = tc.nc
    f = F32
    bf = BF16

    xT = nc.dram_tensor("xT", [D, N], bf, kind="ExternalInput").ap()
    wqkv = nc.dram_tensor("wqkv", [P, KD, CQKV], bf, kind="ExternalInput").ap()
    wo = nc.dram_tensor("wo", [CQ, D], bf, kind="ExternalInput").ap()
    csd = nc.dram_tensor("cs5", [P, NT, 2, CQK], bf, kind="ExternalInput").ap()
    identd = nc.dram_tensor("ident", [P, P], bf, kind="ExternalInput").ap()
    yT = nc.dram_tensor("yT", [D, N], bf, kind="ExternalOutput").ap()

    # ---------------- persistent pools ----------------
    constp = ctx.enter_context(tc.tile_pool(name="const", bufs=1))
    epsb = constp.tile([P, 1], f, tag="epsb")
    nc.vector.memset(epsb[:], EPS)

    persist = ctx.enter_context(tc.tile_pool(name="persist", bufs=1))
    QT2 = persist.tile([P, 2, N], bf, tag="QT2")
    kT2 = persist.tile([P, N], bf, tag="kT2")
    # duplicate of K^T on partitions 64:128, ONE TILE PER KEY CHUNK:
    # DMA-write -> read dependencies are tile-granular, so a shared tile
    # would make every score matmul wait for the LAST dup DMA
    kTd = persist.tile([P, N], bf, tag="kTd")
    v_nat = persist.tile([P, NT, HD + 1], bf, tag="v_nat")
    nc.vector.memset(v_nat[:, :, HD:HD + 1], 1.0)
    O_nat = persist.tile([P, NT, CQ], bf, tag="O_nat")
    oT = persist.tile([P, 2, N], bf, tag="oT")
    cs_all = persist.tile([P, NT, 2, CQK], bf, tag="cs_all")
    ssk_all = persist.tile([P, NT], f, tag="ssk_all")
    rik_all = persist.tile([P, NT], f, tag="rik_all")
    ssq_all = persist.tile([P, NT, 4], f, tag="ssq_all")
    riq_all = persist.tile([P, NT, 4], f, tag="riq_all")

    # ---- input DMA: sync queue = weights + xT; scalar queue = cos/sin ----
    xwp = ctx.enter_context(tc.tile_pool(name="xw", bufs=1))
    w_sb = xwp.tile([P, KD, CQKV], bf, tag="wqkv")
    nc.sync.dma_start(out=w_sb[:], in_=wqkv)
    # cos/sin host-packed per-partition-contiguous (20KB rows): one DMA
    nc.gpsimd.dma_start(out=cs_all[:], in_=csd)
    # xT: even k-chunks on sync, odd on scalar, quarter-major.  SEPARATE
    # tiles per queue: cross-queue writes into one tile serialize through
    # Tile's (tile-granular) dependency tracking, adding ~1.5-3us per DMA.
    xT_e = xwp.tile([P, KD // 2, N], bf, tag="xT_e")
    xT_o = xwp.tile([P, KD // 2, N], bf, tag="xT_o")
    xT_t = xT.rearrange("(k p) n -> k p n", p=P)

    def _xt(k):
        return (xT_e if k % 2 == 0 else xT_o)[:, k // 2, :]

    for k in range(KD):
        eng = nc.sync if k % 2 == 0 else nc.scalar
        eng.dma_start(out=_xt(k), in_=xT_t[k])
    ident = constp.tile([P, P], bf, tag="ident")
    nc.sync.dma_start(out=ident[:], in_=identd)
    wo_sb = constp.tile([P, 2, D], bf, tag="wo")
    wo_t = wo.rearrange("(k p) d -> k p d", p=P)
    for k2 in range(2):
        nc.scalar.dma_start(out=wo_sb[:, k2, :], in_=wo_t[k2])

    # B-phase pools enter FIRST (tile pools are a strict LIFO stack; these
    # outlive the KV/Q pools).  PSUM budget: psS 4 + psO 2 = 6 banks held
    # throughout; KV/Q phases use the remaining 2.
    scale = float(HD) ** -0.5
    yT_t = yT.rearrange("(t p) n -> t p n", p=P)
    otp = ctx.enter_context(tc.tile_pool(name="ot", bufs=9))
    rrp = ctx.enter_context(tc.tile_pool(name="rr", bufs=2))
    ysbp = ctx.enter_context(tc.tile_pool(name="ysb", bufs=9))
    es_b = ExitStack()
    psS = es_b.enter_context(tc.tile_pool(name="psS", bufs=2, space="PSUM"))
    psO = es_b.enter_context(tc.tile_pool(name="psO", bufs=1, space="PSUM"))
    pTp = es_b.enter_context(tc.tile_pool(name="pT", bufs=4))
    es_qsb = ExitStack()
    qnp = es_qsb.enter_context(tc.tile_pool(name="qnp", bufs=6))
    smq = es_qsb.enter_context(tc.tile_pool(name="smq", bufs=4))

    kn_t = {}
    qn_t = {}
    pq_t = {}
    otm = {}
    ysb_tiles = {}

    def q_lnexp_job(g):
        gsl = slice(g * 4, g * 4 + 4)
        lnq = smq.tile([P, 4, 4], f, tag="lnq", name="lnq")
        nc.scalar.activation(lnq[:], ssq_all[:, gsl, :], AF.Ln,
                             bias=epsb[:], scale=1.0 / HD)
        nc.scalar.activation(riq_all[:, gsl, :], lnq[:], AF.Exp, scale=-0.5)

    def q_rope_job(i, pool, tag):
        nsl = slice(i * P, (i + 1) * P)
        qn = qn_t.pop(i)
        qc = smq.tile([P, CQ], bf, tag="qc", name="qc")
        nc.gpsimd.tensor_tensor(qc[:], qn[:], cs_all[:, i, 0, 0:CQ], op=ALU.mult)
        rsq = smq.tile([P, CQ], bf, tag="rsq", name="rsq")
        qn_sw = qn.rearrange("p (j s) -> p j s", s=2)[:, :, ::-1]
        nc.gpsimd.tensor_tensor(rsq[:], qn_sw, cs_all[:, i, 1, 0:CQ], op=ALU.mult)
        qf = smq.tile([P, CQ], bf, tag="qf", name="qf")
        nc.vector.tensor_tensor(qf[:], qc[:], rsq[:], op=ALU.add)
        qf4 = qf.rearrange("p (h d) -> p h d", d=HD)
        nc.vector.tensor_mul(qf4, qf4, riq_all[:, i, :].to_broadcast((P, 4, HD)))
        ptq = pool.tile([P, CQ], bf, tag=tag, name="ptq")
        nc.tensor.transpose(ptq[:, 0:P], qf[:, 0:P], ident[:])
        nc.tensor.transpose(ptq[:, P:CQ], qf[:, P:CQ], ident[:])
        nc.vector.tensor_copy(
            QT2[:, :, nsl], ptq.rearrange("p (h n) -> p h n", h=2))

    def emit_unit(hh, pr, qq, mc, ops):
        qcol = hh * NH + qq * 512
        msl = slice(mc * P, (mc + 1) * P)
        sps = psS.tile([P, 1024], f, tag="psS", name="sps")
        nc.tensor.matmul(
            sps[:, 0:512],
            lhsT=kT2[0:CK, msl],
            rhs=QT2[0:CK, pr, qcol:qcol + 512],
            start=True, stop=True, tile_position=(0, 0),
        )
        nc.tensor.matmul(
            sps[:, 512:1024],
            lhsT=kTd[CK:2 * CK, msl],
            rhs=QT2[CK:2 * CK, pr, qcol:qcol + 512],
            start=True, stop=True, tile_position=(CK, 0),
        )
        pTt = pTp.tile([P, 1024], bf, tag="pT", name="pTt")
        nc.scalar.activation(pTt[:], sps[:], AF.Exp, scale=scale)
        for s in range(2):
            nc.tensor.matmul(
                ops[:, s, :],
                lhsT=v_nat[:, mc, :],
                rhs=pTt[:, s * 512:(s + 1) * 512],
                start=(mc == 0),
                stop=(mc == NT - 1),
            )

    # ================ phase KV: K/V proj + K norm/rope + K^T ================
    # Quarter (0,0,0)'s attention units are WOVEN into KV groups 1-3: unit
    # mc is emitted right after KV tile mc+4 finishes its K^T, so the exp
    # stream starts while the (DMA-gated) KV pass is still running.
    with tc.tile_pool(name="psKV", bufs=1, space="PSUM") as psKV, \
         tc.tile_pool(name="psTK", bufs=1, space="PSUM") as psTK, \
         tc.tile_pool(name="knp", bufs=6) as knp, \
         tc.tile_pool(name="smk", bufs=4) as smk:
        for g in range(4):
            tiles = range(g * 4, g * 4 + 4)
            for i in tiles:
                nsl = slice(i * P, (i + 1) * P)
                # tiles 0-3: fused full-width QKV projection (Q lands here
                # too, so qgroup(0) needs no projection of its own)
                fused = i < 4
                wlo = 0 if fused else CQ
                pkv = psKV.tile([P, CQKV], f, tag="pkv", name="pkv")
                for k in range(KD):
                    nc.tensor.matmul(
                        pkv[:, 0:CQKV - wlo],
                        lhsT=_xt(k)[:, nsl], rhs=w_sb[:, k, wlo:CQKV],
                        start=(k == 0), stop=(k == KD - 1),
                    )
                kn = knp.tile([P, CK], bf, tag="kn", name="kn")
                kn_t[i] = kn
                nc.vector.tensor_copy(kn[:], pkv[:, CQ - wlo:CQ - wlo + CK])
                nc.vector.tensor_copy(v_nat[:, i, 0:HD],
                                      pkv[:, CQ - wlo + CK:CQKV - wlo])
                if fused:
                    qn = qnp.tile([P, CQ], bf, tag="qn", name="qn")
                    qn_t[i] = qn
                    nc.vector.tensor_copy(qn[:], pkv[:, 0:CQ])
                    sq = smk.tile([P, CQ], bf, tag="sq", name="sq")
                    nc.gpsimd.tensor_tensor(sq[:], qn[:], qn[:], op=ALU.mult)
                    nc.vector.tensor_reduce(
                        ssq_all[:, i, :],
                        sq.rearrange("p (h d) -> p h d", d=HD),
                        axis=AX.X, op=ALU.add,
                    )
                sqk = smk.tile([P, CK], bf, tag="sqk")
                nc.vector.tensor_tensor(sqk[:], kn[:], kn[:], op=ALU.mult)
                nc.vector.tensor_reduce(
                    ssk_all[:, i:i + 1],
                    sqk.rearrange("p (h d) -> p h d", d=CK), axis=AX.X, op=ALU.add,
                )
            gsl = slice(g * 4, g * 4 + 4)
            lnk = smk.tile([P, 4], f, tag="lnk")
            nc.scalar.activation(lnk[:], ssk_all[:, gsl], AF.Ln,
                                 bias=epsb[:], scale=1.0 / HD)
            nc.scalar.activation(rik_all[:, gsl], lnk[:], AF.Exp, scale=-0.5)
            for i in tiles:
                nsl = slice(i * P, (i + 1) * P)
                kn = kn_t.pop(i)
                kc = smk.tile([P, CK], bf, tag="kc")
                nc.gpsimd.tensor_tensor(kc[:], kn[:], cs_all[:, i, 0, 2 * P:CQK],
                                        op=ALU.mult)
                rsk = smk.tile([P, CK], bf, tag="rsk")
                kn_sw = kn.rearrange("p (j s) -> p j s", s=2)[:, :, ::-1]
                nc.gpsimd.tensor_tensor(rsk[:], kn_sw, cs_all[:, i, 1, 2 * P:CQK],
                                        op=ALU.mult)
                kf = smk.tile([P, CK], bf, tag="kf")
                nc.vector.tensor_tensor(kf[:], kc[:], rsk[:], op=ALU.add)
                kf1 = kf.rearrange("p (h d) -> p h d", d=CK)
                nc.vector.tensor_mul(
                    kf1, kf1, rik_all[:, i:i + 1].to_broadcast((P, 1, CK)))
                ptk = psTK.tile([P, CQ], bf, tag="ptk", name="ptk")
                nc.tensor.transpose(ptk[0:CK, 0:P], kf[:], ident[:])
                nc.vector.tensor_copy(kT2[0:CK, nsl], ptk[0:CK, 0:P])
                # duplicate this K^T tile onto partitions 64:128 (gpsimd q)
                nc.gpsimd.dma_start(out=kTd[CK:2 * CK, nsl],
                                    in_=kT2[0:CK, nsl])
                if g >= 1:
                    emit_unit(0, 0, 0, i - 4, opsA)
            if g == 0:
                # group-0 Q rope (projection was fused); enables quarter A
                q_lnexp_job(0)
                for i in tiles:
                    q_rope_job(i, psTK, "ptk")
                opsA = psO.tile([HD + 1, 2, 512], f, tag="psO", name="opsA")

    # quarter A's last 4 units + epilogue copy
    for mc in range(12, NT):
        emit_unit(0, 0, 0, mc, opsA)
    o_tmpA = otp.tile([HD + 1, 2, 512], bf, tag="o_tmp", name="o_tmpA")
    nc.vector.tensor_copy(o_tmpA[:], opsA[:])
    otm[(0, 0, 0)] = o_tmpA

    # ---------------- Q pass (grouped; groups 1-3 striped into phase B) ----
    es_q = ExitStack()
    psQ = es_q.enter_context(tc.tile_pool(name="psQ", bufs=1, space="PSUM"))
    psTQ = es_q.enter_context(tc.tile_pool(name="psTQ", bufs=1, space="PSUM"))

    def q_tile_job_a(i):
        nsl = slice(i * P, (i + 1) * P)
        pq = psQ.tile([P, CQ], f, tag="pq", name="pq")
        pq_t[i] = pq
        for k in range(4):
            nc.tensor.matmul(
                pq[:], lhsT=_xt(k)[:, nsl], rhs=w_sb[:, k, 0:CQ],
                start=(k == 0), stop=False,
            )

    def q_tile_job_b(i):
        nsl = slice(i * P, (i + 1) * P)
        pq = pq_t.pop(i)
        for k in range(4, KD):
            nc.tensor.matmul(
                pq[:], lhsT=_xt(k)[:, nsl], rhs=w_sb[:, k, 0:CQ],
                start=False, stop=(k == KD - 1),
            )
        qn = qnp.tile([P, CQ], bf, tag="qn", name="qn")
        qn_t[i] = qn
        nc.vector.tensor_copy(qn[:], pq[:])
        sq = smq.tile([P, CQ], bf, tag="sq", name="sq")
        nc.vector.tensor_tensor(sq[:], qn[:], qn[:], op=ALU.mult)
        nc.vector.tensor_reduce(
            ssq_all[:, i, :],
            sq.rearrange("p (h d) -> p h d", d=HD), axis=AX.X, op=ALU.add,
        )

    def qgroup_jobs(g):
        tiles = list(range(g * 4, g * 4 + 4))
        jobs = []
        for i in tiles:
            jobs.append(lambda i=i: q_tile_job_a(i))
            jobs.append(lambda i=i: q_tile_job_b(i))
        jobs.append(lambda: q_lnexp_job(g))
        jobs += [(lambda i=i: q_rope_job(i, psTQ, "ptq")) for i in tiles]
        return jobs

    # ================ phase B ================

    def attn_quarter(hh, pr, qq, side=()):
        # side: list of closures, one emitted after each unit's instructions
        # so striped work interleaves finely with the ACT-bound exp stream.
        side = list(side)
        ops = psO.tile([HD + 1, 2, 512], f, tag="psO", name="ops")
        for mc in range(NT):
            emit_unit(hh, pr, qq, mc, ops)
            if mc < len(side):
                side[mc]()
        for jb in side[NT:]:
            jb()
        o_tmp = otp.tile([HD + 1, 2, 512], bf, tag="o_tmp", name="o_tmp")
        nc.vector.tensor_copy(o_tmp[:], ops[:])
        otm[(hh, pr, qq)] = o_tmp

    def finisher(hh, pr, qq, scrp):
        # O'^T (SBUF) -> natural via PE transpose, normalize by 1/denom
        o_tmp = otm.pop((hh, pr, qq))
        tbase = hh * 8 + qq * 4
        for s in range(2):
            scr = scrp.tile([P, 4, 68], bf, tag="scr", name="scr")
            for qb in range(4):
                nc.tensor.transpose(
                    scr[:, qb, 0:HD + 1],
                    o_tmp[:, s, qb * P:(qb + 1) * P],
                    ident[0:HD + 1, 0:HD + 1],
                )
            rin = rrp.tile([P, 4], f, tag="rin", name="rin")
            nc.vector.reciprocal(rin[:], scr[:, :, HD])
            h = 2 * pr + s
            nc.vector.tensor_mul(
                O_nat[:, tbase:tbase + 4, h * HD:(h + 1) * HD],
                scr[:, :, 0:HD],
                rin.to_broadcast((P, 4, HD)),
            )

    def b2_job(j, k2, scrp):
        scr = scrp.tile([P, 4, 68], bf, tag="scr", name="scr")
        ptu = scr.rearrange("p a b -> p (a b)")[:, 0:P]
        nc.tensor.transpose(ptu, O_nat[:, j, k2 * P:(k2 + 1) * P], ident[:])
        nc.vector.tensor_copy(oT[:, k2, j * P:(j + 1) * P], ptu)

    def c_job(hh, mt, sg, yp, on_scalar, dma_eng):
        yps = yp.tile([P, 512], f, tag="yps", name="yps")
        qsl = slice(hh * NH + sg * 512, hh * NH + (sg + 1) * 512)
        for k2 in range(2):
            nc.tensor.matmul(
                yps[:],
                lhsT=wo_sb[:, k2, mt * P:(mt + 1) * P],
                rhs=oT[:, k2, qsl],
                start=(k2 == 0),
                stop=(k2 == 1),
            )
        if sg == 0:
            ysb_tiles[(hh, mt)] = ysbp.tile([P, 2, 512], bf, tag="ysb",
                                            name="ysb")
        ysb = ysb_tiles[(hh, mt)]
        if on_scalar:
            nc.scalar.copy(ysb[:, sg, :], yps[:])
        else:
            nc.vector.tensor_copy(ysb[:, sg, :], yps[:])
        if sg == 1:
            dma_eng.dma_start(
                out=yT_t[mt][:, hh * NH:(hh + 1) * NH],
                in_=ysb.rearrange("p a b -> p (a b)"),
            )

    # ---- schedule: quarters qq-outer so each striped Q group has a full
    # quarter of slack before its output is consumed; hh0's epilogue work
    # striped into late-hh0/hh1 quarters; hh1's qq0 epilogue striped into
    # the qq1 quarters; only the qq1 epilogue remains as a true tail. ----
    # quarter (0,0,0) already ran, woven into the KV pass
    attn_quarter(0, 1, 0, side=qgroup_jobs(1))            # tiles 4-7 (for C)
    attn_quarter(0, 0, 1, side=qgroup_jobs(2) + qgroup_jobs(3))
    es_q.close()
    es_qsb.close()
    es_s = ExitStack()
    psScr = es_s.enter_context(tc.tile_pool(name="psScr", bufs=1, space="PSUM"))
    psY = es_s.enter_context(tc.tile_pool(name="psY", bufs=1, space="PSUM"))

    side_D = ([lambda: finisher(0, 0, 0, psScr), lambda: finisher(0, 1, 0, psScr)]
              + [(lambda j=j, k2=k2: b2_job(j, k2, psScr))
                 for j in range(0, 4) for k2 in range(2)])
    attn_quarter(0, 1, 1, side=side_D)

    side_E = ([lambda: finisher(0, 0, 1, psScr), lambda: finisher(0, 1, 1, psScr)]
              + [(lambda j=j, k2=k2: b2_job(j, k2, psScr))
                 for j in range(4, 8) for k2 in range(2)]
              + [(lambda sg=sg: c_job(0, 0, sg, psY, False, nc.sync))
                 for sg in range(2)])
    attn_quarter(1, 0, 0, side=side_E)

    side_F = [(lambda mt=mt, sg=sg: c_job(0, mt, sg, psY, False, nc.sync))
              for mt in range(1, 8) for sg in range(2)]
    attn_quarter(1, 1, 0, side=side_F)

    side_G = ([lambda: finisher(1, 0, 0, psScr), lambda: finisher(1, 1, 0, psScr)]
              + [(lambda j=j, k2=k2: b2_job(j, k2, psScr))
                 for j in range(8, 12) for k2 in range(2)])
    attn_quarter(1, 0, 1, side=side_G)

    side_H = ([(lambda mt=mt: c_job(1, mt, 0, psY, False, nc.sync))
               for mt in range(8)]
              + [lambda: finisher(1, 0, 1, psScr)]
              + [(lambda j=j: b2_job(j, 0, psScr)) for j in range(12, 16)])
    attn_quarter(1, 1, 1, side=side_H)

    # ---- tail: remaining hh1/qq1 epilogue with fat psum pools ----
    es_s.close()
    es_b.close()
    with tc.tile_pool(name="psScr2", bufs=3, space="PSUM") as psScr2, \
         tc.tile_pool(name="psY2", bufs=4, space="PSUM") as psY2:
        finisher(1, 1, 1, psScr2)
        for j in range(12, 16):
            b2_job(j, 1, psScr2)
        for mt in range(8):
            c_job(1, mt, 1, psY2, on_scalar=(mt % 2 == 0),
                  dma_eng=(nc.scalar if mt % 2 else nc.sync))


def _patch_act_tables(arch):
    """Reorder the (process-cached) activation-table dict so Exp, Ln and
    Copy all resolve to the single `natural_log_exp_and_others` set.  The
    default greedy per-function selection puts Exp in `exp_and_others` and
    Ln in `natural_log`, reloading ACT tables (~2.7us each) every time an
    RMSNorm Ln/Exp pair is interleaved with the softmax Exp stream."""
    from concourse.hw_specs import get_activation_tables
    t = get_activation_tables(arch)
    pref = "natural_log_exp_and_others"
    # Keep dict order/keys intact (set indices must match the real
    # act_info.json); just make `pref` the only set containing Exp/Ln.
    for name, fns in t.items():
        if name != pref:
            fns.discard(AF.Exp)
            fns.discard(AF.Ln)


def _build_program():
    global _PROGRAM
    if _PROGRAM is not None:
        return _PROGRAM
    nc = bacc.Bacc(
        "TRN2",
        target_bir_lowering=False,
        debug=False,
        enable_asserts=False,
        num_devices=NCORES,
    )
    _patch_act_tables(nc.m.arch)
    with tile.TileContext(nc) as tc:
        with ExitStack() as ctx:
            _emit(ctx, tc)
    nc.finalize()
    _PROGRAM = nc
    return nc


# ---------------- host-side RoPE/scale table construction ----------------

def _rope_cs(n, d):
    inv = 1.0 / (THETA ** (np.arange(0, d, 2, dtype=np.float32) / d))
    fr = np.arange(n, dtype=np.float32)[:, None] * inv[None, :]
    emb = np.concatenate([fr, fr], axis=-1)
    return np.cos(emb), np.sin(emb)


def _perm():
    dt = HD // 4
    dh = HD // 4
    dw = HD - dt - dh
    perm = np.empty(HD, np.int64)
    for off, sz in ((0, dt), (dt, dh), (dt + dh, dw)):
        m = sz // 2
        for j in range(m):
            perm[off + 2 * j] = off + j
            perm[off + 2 * j + 1] = off + m + j
    return perm


def _host_tables(qn_w, kn_w):
    dt = HD // 4
    dh = HD // 4
    dw = HD - dt - dh
    cos_t, sin_t = _rope_cs(T_, dt)
    cos_h, sin_h = _rope_cs(H_, dh)
    cos_w, sin_w = _rope_cs(W_, dw)
    tt = np.repeat(np.arange(T_), H_ * W_)
    hh = np.tile(np.repeat(np.arange(H_), W_), T_)
    ww = np.tile(np.arange(W_), T_ * H_)
    cos = np.empty((N, HD), np.float32)
    sin = np.empty((N, HD), np.float32)
    cos[:, 0:dt] = cos_t[tt]
    cos[:, dt:dt + dh] = cos_h[hh]
    cos[:, dt + dh:] = cos_w[ww]
    sin[:, 0:dt] = sin_t[tt]
    sin[:, dt:dt + dh] = sin_h[hh]
    sin[:, dt + dh:] = sin_w[ww]

    perm = _perm()
    cosP = cos[:, perm]
    sgn = np.empty(HD, np.float32)
    sgn[0::2] = -1.0
    sgn[1::2] = 1.0
    sgnsinP = sin[:, perm] * sgn[None, :]

    def fold(w):
        wp = np.asarray(w, np.float32)[perm]
        swp = wp.reshape(-1, 2)[:, ::-1].reshape(-1)
        return cosP * wp[None, :], sgnsinP * swp[None, :]

    cos_q, sin_q = fold(qn_w)
    cos_k, sin_k = fold(kn_w)
    cos5 = np.concatenate([np.tile(cos_q, (1, 4)), cos_k], axis=1)
    sin5 = np.concatenate([np.tile(sin_q, (1, 4)), sin_k], axis=1)
    return np.ascontiguousarray(cos5), np.ascontiguousarray(sin5)


def _bf16(a):
    return np.asarray(a, np.float32).astype(ml_dtypes.bfloat16)


def kernel(**inputs):
    global LAST_RESULTS
    x = np.asarray(inputs["x"], np.float32)
    Wq = np.asarray(inputs["Wq"], np.float32)
    Wk = np.asarray(inputs["Wk"], np.float32)
    Wv = np.asarray(inputs["Wv"], np.float32)
    Wo = np.asarray(inputs["Wo"], np.float32)
    bq = np.asarray(inputs["bq"], np.float32)
    bk = np.asarray(inputs["bk"], np.float32)
    bv = np.asarray(inputs["bv"], np.float32)
    bo = np.asarray(inputs["bo"], np.float32)
    qn_w = np.asarray(inputs["qn_w"], np.float32)
    kn_w = np.asarray(inputs["kn_w"], np.float32)

    assert x.shape == (B, N, D), x.shape
    # device program omits the qkv bias add (biases are zero in this model)
    assert not bq.any() and not bk.any() and not bv.any(), \
        "nonzero qkv bias unsupported by this kernel build"
    cos5, sin5 = _host_tables(qn_w, kn_w)
    perm = _perm()

    nc = _build_program()
    in_maps = []
    xT_b = [np.ascontiguousarray(_bf16(x[b]).T) for b in range(B)]
    identb = np.eye(P, dtype=ml_dtypes.bfloat16)
    csH = np.stack([cos5.reshape(NT, P, CQK), sin5.reshape(NT, P, CQK)],
                   axis=2).transpose(1, 0, 2, 3)   # [P, NT, 2, CQK]
    cs5b = np.ascontiguousarray(_bf16(csH))
    for c in range(NCORES):
        b, g = c // 4, c % 4
        wq_g = Wq[:, g * CQ:(g + 1) * CQ].reshape(D, 4, HD)[:, :, perm].reshape(D, CQ)
        wk_g = Wk[:, g * CK:(g + 1) * CK][:, perm]
        wv_g = Wv[:, g * CK:(g + 1) * CK]
        wqkv_ = np.concatenate([wq_g, wk_g, wv_g], axis=1)
        wH = _bf16(wqkv_).reshape(KD, P, CQKV).transpose(1, 0, 2)
        in_maps.append({
            "xT": xT_b[b],
            "wqkv": np.ascontiguousarray(wH),
            "wo": np.ascontiguousarray(_bf16(Wo[g * CQ:(g + 1) * CQ, :])),
            "cs5": cs5b,
            "ident": identb,
        })

    res = run_bass_kernel_spmd(nc, in_maps, list(range(NCORES)))
    LAST_RESULTS = res
    out = np.empty((B, N, D), np.float32)
    for b in range(B):
        acc = res.results[4 * b]["yT"].astype(np.float32)
        for g in range(1, 4):
            acc = acc + res.results[4 * b + g]["yT"].astype(np.float32)
        out[b] = acc.T + bo[None, :]
    return out


if __name__ == "__main__":
    nc = _build_program()
    print("built ok")
